# revision 45
# baseline (speedup 1.0000x reference)
"""CostVolumeLayer Trainium2 kernel (banded-gram v2).

Problem: src, tgt [B=8, C=128, H=160, W=288] fp32.
out[b, k, y, x] = (1/C) * sum_c src[b,c,y,x] * tgt[b,c,y+dy_k,x+dx_k]
for the 81 displacements (dy,dx) in [-4,4]^2 (torch CostVolume channel order),
with zero padding outside the image.

Strategy (data-parallel over batch, one batch per NeuronCore):
  - Position tiles of TY x TX = 128 positions (partition dim). For each tile
    the PE computes the Gram block src_tile[C,128].T @ tgt_window[C, WIN_Y*WIN_X]
    with ONE matmul streaming a 2-D strided rhs AP (WIN_Y rows x WIN_X cols of
    the resident padded tgt), split only where the window crosses a tgt chunk
    tile boundary. Two tiles' matmuls share one 2-bank PSUM tile.
  - DVE/ACT alternate evacuating both banks per op, PSUM -> SBUF strip buffer
    laid out [p, wy, t, wx] (x 1/C, cast bf16), so each position-row group
    ty's needed band (window rows [ty, ty+9)) is one contiguous per-partition
    byte range spanning all tiles of the strip.
  - Per (strip, ty) one DMA writes the [TX partitions x 9*NXT*WIN_X] band to
    DRAM: output traffic is 9*WIN_X/81 = (TX+8)/9 x the true output (13.3MB)
    instead of WIN_Y*WIN_X/81 ~ 4.7x (44.2MB).
  - The host de-shears the bands into [B, 81, H, W] (cheap numpy gather; the
    dx shear is not expressible as any engine/DMA access pattern on-device).
  - Inputs are cast to bf16 on host (halves HBM read); PSUM accumulates fp32.
  - Ring discipline (load-bearing for pipelining): SP carries the startup
    loads (split chunk-0 + split src strip 0, highest priority) and most
    band DMAs; ACT carries steady-state input prefetch (src strips + tgt
    chunks, issued at strip tops ahead of ACT's evac ops so they never queue
    behind band DMAs, whose semaphores wait on evacuation); Pool (SWDGE)
    carries the remaining band DMAs; DVE does evacuation only.
"""

import sys

for _p in ("/opt/trn_rl_repo",):
    if _p not in sys.path:
        sys.path.insert(0, _p)

import numpy as np
import ml_dtypes

import concourse.mybir as mybir
import concourse.tile as tile
from concourse import bacc
from concourse.bass_utils import run_bass_kernel_spmd

B, C, H, W, S = 8, 128, 160, 288, 4
TY, TX = 16, 8                       # position tile = TY x TX = 128 positions
WIN_Y, WIN_X = TY + 2 * S, TX + 2 * S
NWIN = WIN_Y * WIN_X                 # PSUM cols per tile (fp32, <=512)
NSTRIP = H // TY
NXT = W // TX
HP, WP = H + 2 * S, W + 2 * S        # padded tgt dims
TGT_CHUNK = 24                       # tgt rows per resident chunk tile
N_TGT_CHUNKS = (HP + TGT_CHUNK - 1) // TGT_CHUNK
NBAND = 9 * WIN_X                    # band cols per position (9 window rows)
STRIPW = NXT * WIN_X                 # per-wy row size in strip buffer (elems)
NP0 = 4                              # chunk-0 / src-0 startup pieces
P0T = NXT // NP0                     # tiles per piece (9)
P0W = 88                             # piece cols (covers P0T tiles + halo)
P0STEP = P0T * TX                    # col offset step between pieces (72)
N_CORES = 8

assert TY * TX == 128 and H % TY == 0 and W % TX == 0
assert NWIN * 4 <= 2048, "psum tile must fit one bank"
assert HP <= N_TGT_CHUNKS * TGT_CHUNK

BF16 = mybir.dt.bfloat16
NP_BF16 = ml_dtypes.bfloat16


def _displacements(s):
    d = [(0, 0)]
    for i in range(1, s + 1):
        d += [(-i, 0), (i, 0), (0, -i), (0, i)]
        for j in range(1, s + 1):
            d += [(-i, -j), (i, j), (-i, j), (i, -j)]
    return d


DISPLACEMENTS = _displacements(S)


def _row_segments(r0, r1):
    """Split padded-tgt row range [r0, r1) at TGT_CHUNK boundaries.

    Yields (wy_off, chunk_idx, row_in_chunk, nrows)."""
    segs = []
    r = r0
    while r < r1:
        c = r // TGT_CHUNK
        hi = min(r1, (c + 1) * TGT_CHUNK)
        segs.append((r - r0, c, r - c * TGT_CHUNK, hi - r))
        r = hi
    return segs


def _build_bass():
    nc = bacc.Bacc(
        "TRN2",
        target_bir_lowering=False,
        debug=False,
        num_devices=N_CORES,
    )
    # src pre-tiled on host: [C, NSTRIP, NXT*128] so each tile's lhsT is one
    # contiguous 128-element slice (position p = ty*TX + tx).
    src_t = nc.dram_tensor(
        "src", [C, NSTRIP, NXT * 128], BF16, kind="ExternalInput"
    ).ap()
    tgt_t = nc.dram_tensor("tgtp", [C, HP, WP], BF16, kind="ExternalInput").ap()
    # chunk-0 column pieces pre-split by the host into a contiguous DRAM
    # tensor (piece j = cols [72j, 72j+88) of the first 24 padded rows): the
    # on-device loads are full-rate contiguous descriptors, and interleaving
    # them with src-0 pieces lets the PE start ~5us earlier and stream.
    tgt0_t = nc.dram_tensor(
        "tgt0p", [C, NP0, TGT_CHUNK * P0W], BF16, kind="ExternalInput"
    ).ap()
    band_t = nc.dram_tensor(
        "band", [NSTRIP, TY, TX, 9 * STRIPW], BF16, kind="ExternalOutput"
    ).ap()

    with tile.TileContext(nc) as tc:
        with (
            tc.tile_pool(name="tgt0", bufs=1) as tgt0_pool,
            tc.tile_pool(name="tgtres", bufs=3) as tgt_pool,
            tc.tile_pool(name="srcstrip", bufs=3) as src_pool,
            tc.tile_pool(name="bandstrip", bufs=3) as band_pool,
            tc.tile_pool(name="psum", bufs=4, space="PSUM") as psum_pool,
        ):
            # tgt resident in SBUF in row chunks (separate tiles) so early
            # strips' matmuls depend only on the first chunks. Chunk 0 is
            # split into two column halves loaded FIRST on the SP ring
            # (which spins up earlier than ACT) so strip 0's matmuls start
            # ~8us sooner; chunks 1+ stream on ACT.
            c0p, c0p_v = [], []
            for j in range(NP0):
                c0piece = tgt0_pool.tile([C, TGT_CHUNK * P0W], BF16, tag=f"tgtc0p{j}")
                c0p.append(c0piece)
                c0p_v.append(c0piece.rearrange("p (y x) -> p y x", x=P0W))

            # chunks 1+ cycle through a 4-buffer pool; issued lazily (two
            # strips ahead of first use) so the WAR wait on a recycled
            # buffer never blocks ACT's evacuation stream.
            tgt_chunks = [None] * N_TGT_CHUNKS

            def issue_chunk(ci, eng=None):
                rows = min(TGT_CHUNK, HP - ci * TGT_CHUNK)
                ch = tgt_pool.tile([C, rows * WP], BF16)
                (eng or nc.sync).dma_start(
                    ch[:], tgt_t[:, ci * TGT_CHUNK : ci * TGT_CHUNK + rows, :]
                )
                tgt_chunks[ci] = ch.rearrange("p (y x) -> p y x", x=WP)

            def chunk_first_strip(ci):
                # first strip whose window rows [TY*s, TY*s+WIN_Y) touch chunk ci
                return max(0, -(-(TGT_CHUNK * ci - WIN_Y + 1) // TY))

            def chunk_rhs(ci, r0, nr, t):
                """rhs AP for window rows [r0, r0+nr) of chunk ci, tile t."""
                x0 = t * TX
                if ci == 0:
                    j = t // P0T
                    lc = x0 - j * P0STEP
                    return c0p_v[j][:, r0 : r0 + nr, lc : lc + WIN_X]
                return tgt_chunks[ci][:, r0 : r0 + nr, x0 : x0 + WIN_X]

            # src strip loads with prefetch: in SP's program order each
            # src dma_start must come BEFORE the band DMAs that wait on the
            # current strip's evacuation, or input prefetch serializes behind
            # compute (the v2 stop-go pattern). PREFETCH must stay < bufs so
            # the pool-reuse WAR wait is already satisfied at issue time and
            # never blocks SP. Strip 0 is split in half and interleaved with
            # the chunk-0 halves so the first matmul's inputs land early.
            PREFETCH = 2
            src_tiles = {}

            def issue_src(s, eng=None):
                # Warmup src strips ride ACT's ring (issued before any evac
                # exists); steady-state strips ride SP, issued FIRST in each
                # strip iteration — their WAR (matmuls of s-3) is stale by
                # then, so they never block, and ACT stays evac-only.
                st = src_pool.tile([C, NXT * 128], BF16)
                (eng or nc.sync).dma_start(st[:], src_t[:, s])
                v = st.rearrange("p (t m) -> p t m", m=128)
                src_tiles[s] = lambda t, v=v: v[:, t, :]

            # Warmup loads ride SP's queue in need order, chunk-0 piece and
            # src-0 piece alternating, so the PE starts after the first pair
            # (~0.8MB) and streams as later pieces land.
            s0p = []
            for j in range(NP0):
                nc.sync.dma_start(c0p[j][:], tgt0_t[:, j])
                sp = src_pool.tile([C, P0T * 128], BF16, tag=f"src0p{j}")
                nc.sync.dma_start(sp[:], src_t[:, 0, j * P0T * 128 : (j + 1) * P0T * 128])
                s0p.append(sp.rearrange("p (t m) -> p t m", m=128))
            src_tiles[0] = lambda t: s0p[t // P0T][:, t % P0T, :]

            chunk_issue_at = {}
            for ci in range(1, N_TGT_CHUNKS):
                chunk_issue_at.setdefault(
                    max(0, chunk_first_strip(ci) - 2), []
                ).append(ci)
            issue_src(1, eng=nc.scalar)
            for ci in chunk_issue_at.pop(0, []):
                issue_chunk(ci, eng=nc.scalar)
            issue_src(2, eng=nc.scalar)

            # Band DMA rings: SP is cheap (565ns/DMA); Pool SWDGE ~0.75us.
            # DVE/ACT stay evac-only (the PSUM->SBUF evacuation is the other
            # ~80us/engine workload). Keep SP's share small: band DMAs wait
            # on the strip's evacuation, and src prefetch issues queue behind
            # them in SP's FIFO.
            N_SP_BAND = 7

            for s in range(NSTRIP):
                # src(1), src(2) were issued in warmup; with 3 cycling src
                # buffers, src(s+2) at strip s reuses src(s-1)'s buffer whose
                # WAR (strip s-1 matmuls) is already satisfied at issue time.
                if s >= 1 and s + PREFETCH < NSTRIP:
                    issue_src(s + PREFETCH)
                for ci in chunk_issue_at.pop(s, []):
                    issue_chunk(ci)
                src_lhsT = src_tiles.pop(s)

                band_tile = band_pool.tile([C, WIN_Y * STRIPW], BF16)
                # out view for paired evacuation: dims (t, wy, wx) so one op
                # can consume two PSUM banks (two tiles) in bank-major order.
                bt = band_tile.rearrange(
                    "p (wy t wx) -> p t wy wx", t=NXT, wx=WIN_X
                )

                segs = _row_segments(s * TY, s * TY + WIN_Y)
                for tp in range(NXT // 2):
                    # One PSUM tile = 2 banks = 2 position tiles; each matmul
                    # output stays within one bank (cols [0,384)/[512,896)).
                    ps = psum_pool.tile([128, 2 * 512], mybir.dt.float32)
                    for half in range(2):
                        t = 2 * tp + half
                        base = half * 512
                        for (wy0, ci, r0, nr) in segs:
                            nc.tensor.matmul(
                                ps[:, base + wy0 * WIN_X : base + (wy0 + nr) * WIN_X],
                                lhsT=src_lhsT(t),
                                rhs=chunk_rhs(ci, r0, nr, t),
                                start=True,
                                stop=True,
                            )
                    # Evacuate both banks with one op: in dims (bank, 384),
                    # out dims (t-pair, wy, wx) — identical element order.
                    pv = ps.rearrange("p (b r) -> p b r", r=512)[:, :, :NWIN]
                    ov = bt[:, 2 * tp : 2 * tp + 2, :, :]
                    if tp % 2 == 0:
                        nc.vector.tensor_scalar_mul(ov, pv, 1.0 / C)
                    else:
                        nc.scalar.mul(ov, pv, 1.0 / C)

                # Band extraction: per position-row group g (= ty), write the
                # 9 window rows [g, g+9) x all tiles: per-partition contiguous.
                # On the last strip, bias toward SP (cheaper sequencing) to
                # shorten the drain tail.
                n_sp = 11 if s == NSTRIP - 1 else N_SP_BAND
                for g in range(TY):
                    ring = nc.sync if g < n_sp else nc.gpsimd
                    ring.dma_start(
                        band_t[s, g],
                        band_tile[g * TX : (g + 1) * TX, g * STRIPW : (g + 9) * STRIPW],
                    )

    nc.compile()
    return nc


_NC = None


def _get_nc():
    global _NC
    if _NC is None:
        _NC = _build_bass()
    return _NC


def _run_device(src_bf, tgtp_bf, **run_kwargs):
    nc = _get_nc()
    # chunk-0 column pieces: piece j = cols [72j, 72j+88) of rows [0, 24),
    # zero-padded past the image's right edge (never read there).
    t0p = np.zeros((B, C, NP0, TGT_CHUNK, P0W), NP_BF16)
    for j in range(NP0):
        w = min(P0W, WP - j * P0STEP)
        t0p[:, :, j, :, :w] = tgtp_bf[
            :, :, :TGT_CHUNK, j * P0STEP : j * P0STEP + w
        ]
    t0p = t0p.reshape(B, C, NP0, TGT_CHUNK * P0W)
    in_maps = [
        {"src": src_bf[b], "tgtp": tgtp_bf[b], "tgt0p": t0p[b]}
        for b in range(B)
    ]
    return run_bass_kernel_spmd(nc, in_maps, core_ids=list(range(N_CORES)), **run_kwargs)


def _deshear(band):
    """band: [B, NSTRIP, TY, TX, 9, NXT, WIN_X] -> [B, 81, H, W] fp32."""
    band = np.asarray(band, dtype=np.float32)
    out = np.empty((B, len(DISPLACEMENTS), H, W), np.float32)
    txx = np.arange(TX)
    for k, (dy, dx) in enumerate(DISPLACEMENTS):
        # v[b, s, ty, tx, t] = band[b, s, ty, tx, dy+4, t, tx+dx+4]
        v = band[:, :, :, txx, dy + S, :, txx + dx + S]
        # v axes: (tx, b, s, ty, t) -> [B, (s,ty)=H, (t,tx)=W]
        out[:, k] = v.transpose(1, 2, 3, 4, 0).reshape(B, H, W)
    return out


def kernel(src, tgt, _profile_out=None):
    src = np.asarray(src)
    tgt = np.asarray(tgt)
    assert src.shape == (B, C, H, W) and tgt.shape == (B, C, H, W)

    # [B,C,H,W] -> [B,C,NSTRIP,TY,NXT,TX] -> [B,C,NSTRIP,NXT,TY*TX]
    src_bf = np.ascontiguousarray(
        src.astype(NP_BF16)
        .reshape(B, C, NSTRIP, TY, NXT, TX)
        .transpose(0, 1, 2, 4, 3, 5)
        .reshape(B, C, NSTRIP, NXT * TY * TX)
    )
    tgtp_bf = np.zeros((B, C, HP, WP), NP_BF16)
    tgtp_bf[:, :, S : S + H, S : S + W] = tgt.astype(NP_BF16)

    kw = {}
    if _profile_out is not None:
        kw["trace"] = True
    res = _run_device(src_bf, tgtp_bf, **kw)
    if _profile_out is not None:
        _profile_out.update(
            exec_time_ns=res.exec_time_ns,
            mean_exec_time_ns=res.mean_exec_time_ns,
        )

    band = np.stack([res.results[b]["band"] for b in range(B)]).reshape(
        B, NSTRIP, TY, TX, 9, NXT, WIN_X
    )
    return _deshear(band)
